# revision 1
# baseline (speedup 1.0000x reference)
"""Chamfer distance loss kernel for 8 Trainium2 NeuronCores.

Problem: template/source point clouds [B=4, N=8192, 3] fp32.
  d2[b,n,m] = ||t[b,n] - s[b,m]||^2
  out = mean_b( (mean_n sqrt(min_m d2) + mean_m sqrt(min_n d2)) / 2 )

Sharding: core c handles batch b=c//2, template-row half h=c%2.  Unlike
the 2-pass baseline (which recomputed the transposed matrix for the
source->template direction), each core computes its 4096x8192 slab of
the distance matrix ONCE and extracts BOTH directions from it:
  - row minima (template->source): free-axis min per template row
  - column minima partials (source->template): running elementwise min
    across strips, partition-reduced at the end via PE transpose;
    the two cores sharing a batch are combined on the host.

Per-strip pipeline (strip = 128 template rows):
  PE    : 16 matmuls [128,512] fill PSUM (two 4-bank groups,
          double-buffered).  K=14 f32r hi/lo split reproduces exact-fp32
          brackets b2[m] - 2 t.s as in the baseline.
  ACT   : one activation per 4-bank group: Relu(psum + a2[row]) -> bf16
          strip buffer in SBUF (adds |t|^2, clamps at 0, narrows to 16
          bits so the DVE can run its 2x packed mode).
  DVE   : one tensor_reduce (min) over the [128,8192] bf16 strip ->
          rowmin, and one tensor_tensor (min) folding the strip into the
          running [128,8192] bf16 column-min - both at 2 elem/cyc/lane.

This cuts PE work in half vs the baseline and, more importantly, cuts
the DVE reduction stream from 2x262144 fp32-from-PSUM cycles (the
baseline bottleneck, ~546us at 0.96GHz) to ~262144 bf16-from-SBUF
cycles, with the PSUM drain moved to ACT which runs in parallel.

Epilogue: sqrt+sum of rowmin on device ([128,1] out); column partials
are PE-transposed in 64 [128,128] blocks into PSUM and min-reduced to
[128,64] raw d2 values; host combines core pairs, sqrts, and averages.
"""

import numpy as np

B = 4
N = 8192  # points per cloud
HALF = N // 2  # template rows per core
N_CORES = 8
STRIPS = HALF // 128  # 32
M_TILES = N // 512  # 16
K_ROWS = 14  # hi/lo-split contraction
CBLK = N // 128  # 64 column-min output blocks

_cache = {}


def _build_bass(reps=1, ablate=()):
    """ablate: subset of {'colp','rowred','act','mm','epi'} to drop pieces
    for timing ablation (results are garbage when non-empty)."""
    import contextlib
    from concourse import bacc, mybir, tile, masks

    f32 = mybir.dt.float32
    f32r = mybir.dt.float32r
    bf16 = mybir.dt.bfloat16
    AOp = mybir.AluOpType
    AFT = mybir.ActivationFunctionType

    nc = bacc.Bacc("TRN2", target_bir_lowering=False, debug=False,
                   num_devices=N_CORES)

    lhs = nc.dram_tensor("lhs", [K_ROWS, HALF], f32r,
                         kind="ExternalInput").ap()
    rhs = nc.dram_tensor("rhs", [K_ROWS, N], f32r,
                         kind="ExternalInput").ap()
    a2 = nc.dram_tensor("a2", [128, STRIPS], f32, kind="ExternalInput").ap()
    out_row = nc.dram_tensor("out_row", [128, STRIPS], f32,
                             kind="ExternalOutput").ap()
    out_col = nc.dram_tensor("out_col", [128, CBLK], f32,
                             kind="ExternalOutput").ap()

    with tile.TileContext(nc) as tc:
        with tc.tile_pool(name="const", bufs=1) as cpool, \
             tc.tile_pool(name="psum", bufs=1, space="PSUM") as ppool, \
             tc.tile_pool(name="strip", bufs=3) as spool:

            lhs_sb = cpool.tile([K_ROWS, HALF], f32r, tag="lhs")
            rhs_sb = cpool.tile([K_ROWS, N], f32r, tag="rhs")
            a2_sb = cpool.tile([128, STRIPS], f32, tag="a2")
            ident = cpool.tile([128, 128], f32, tag="ident")
            colp = cpool.tile([128, N], bf16, tag="colp")
            colpf = cpool.tile([128, N], f32, tag="colpf")
            rowmin = cpool.tile([128, STRIPS], f32, tag="rowmin")
            ocol_sb = cpool.tile([128, CBLK], f32, tag="ocol")

            nc.sync.dma_start(lhs_sb[:, :], lhs)
            nc.sync.dma_start(rhs_sb[:, :], rhs)
            nc.sync.dma_start(a2_sb[:, :], a2)
            masks.make_identity(nc, ident[:, :])

            # One PSUM tensor spanning all 8 banks, viewed [128, 32, 128]:
            # matmul tiles are 4 slots, ACT groups 16 slots, and the
            # epilogue reuses it as 32 transposed blocks for one segmented
            # min-reduce.
            P = ppool.tile([128, 32, 128], f32, tag="P")

            loop_ctx = (tc.For_i(0, reps, 1) if reps > 1
                        else contextlib.nullcontext())
            with loop_ctx:
                for s in range(STRIPS):
                    strip_sb = spool.tile([128, N], bf16, tag="strip")
                    for g in range(4):
                        base = 16 * (g % 2)  # PSUM slot of this 4-bank group
                        if "mm" not in ablate:
                            for j in range(4):
                                m = 4 * g + j
                                nc.tensor.matmul(
                                    P[:, base + 4 * j: base + 4 * (j + 1), :],
                                    lhsT=lhs_sb[:, 128 * s: 128 * (s + 1)],
                                    rhs=rhs_sb[:, 512 * m: 512 * (m + 1)],
                                    start=True, stop=True,
                                )
                        if "act" not in ablate:
                            nc.scalar.activation(
                                strip_sb[:, 2048 * g: 2048 * (g + 1)],
                                P[:, base: base + 16, :],
                                AFT.Relu, bias=a2_sb[:, s:s + 1],
                            )
                    # column chain first (needs the strip intact); the row
                    # chain then folds the strip in place.  tensor_tensor
                    # runs at 2 elem/cyc/lane on 16-bit data; tensor_reduce
                    # only manages 1x, so fold 8192->512 with TTs and keep
                    # just a 512-wide reduce tail.
                    if "colp" not in ablate:
                        if s == 0:
                            nc.vector.tensor_copy(colp[:, :], strip_sb[:, :])
                        else:
                            nc.vector.tensor_tensor(
                                colp[:, :], colp[:, :], strip_sb[:, :],
                                AOp.min,
                            )
                    if "rowred" not in ablate:
                        # row-min in ONE 4x-mode instruction: identity
                        # tensor_scalar (min with +inf, written back in
                        # place) whose accumulator reduces with op1=min.
                        nc.vector.tensor_scalar(
                            strip_sb[:, :], strip_sb[:, :],
                            3.0e38, None,
                            AOp.min, AOp.min,
                            accum_out=rowmin[:, s:s + 1],
                        )

                if "epi" not in ablate:
                    # column epilogue: partition-min via PE transpose (fp32:
                    # transpose output dtype must match its input, and PSUM
                    # is fully booked by the fp32 accumulator)
                    # (only PSUM slots 16..31, so the next iteration's
                    # even-group matmuls don't stall behind the epilogue)
                    nc.scalar.activation(colpf[:, :], colp[:, :], AFT.Copy)
                    for r in range(4):
                        for t in range(16):
                            blk = 16 * r + t
                            nc.tensor.transpose(
                                P[:, 16 + t, :],
                                colpf[:, 128 * blk: 128 * (blk + 1)],
                                ident[:, :],
                            )
                        nc.vector.tensor_reduce(
                            ocol_sb[:, 16 * r: 16 * (r + 1)], P[:, 16:32, :],
                            axis=mybir.AxisListType.X, op=AOp.min,
                        )

                nc.sync.dma_start(out_row, rowmin[:, :])
                nc.sync.dma_start(out_col, ocol_sb[:, :])

    nc.compile()
    return nc


def _rnd11(x):
    """Round-to-nearest keeping 11 explicit mantissa bits (the rounding the
    PE applies to float32r operands, measured on HW)."""
    xi = x.view(np.uint32).astype(np.uint64)
    out = ((xi + np.uint64(1 << 11)) & np.uint64(0xFFFFF000)).astype(np.uint32)
    return out.view(np.float32)


def _hilo(x):
    hi = _rnd11(np.ascontiguousarray(x, np.float32))
    lo = _rnd11((x - hi).astype(np.float32))
    return hi, lo


def _sq(x):  # |x|^2 per point, fp32
    return (x * x).sum(axis=-1, dtype=np.float32)


def _prep_core_inputs(template, source, c):
    b, h = divmod(c, 2)
    tch = template[b, h * HALF:(h + 1) * HALF]  # [4096, 3] rows
    sfull = source[b]  # [8192, 3] cols

    # stationary operand: [14, 4096] = hi/lo split of -2*t
    v = (-2.0 * tch.T).astype(np.float32)  # [3, n]
    ones = np.ones((1, HALF), np.float32)
    vh, vl = _hilo(v)
    lhs = np.ascontiguousarray(
        np.concatenate([vh, vh, vl, vl, ones, ones], axis=0))

    # moving operand: [14, 8192] = hi/lo split of s plus |s|^2 rows
    w = np.ascontiguousarray(sfull.T, np.float32)  # [3, m]
    b2 = _sq(sfull)[None]  # [1, m]
    wh, wl = _hilo(w)
    b2h, b2l = _hilo(b2)
    rhs = np.ascontiguousarray(
        np.concatenate([wh, wl, wh, wl, b2h, b2l], axis=0))

    return {
        "lhs": lhs,
        "rhs": rhs,
        "a2": np.ascontiguousarray(_sq(tch).reshape(STRIPS, 128).T),
    }


def _run(template, source, trace=False):
    from concourse.bass_utils import run_bass_kernel_spmd

    template = np.asarray(template, np.float32)
    source = np.asarray(source, np.float32)
    assert template.shape == (B, N, 3) and source.shape == (B, N, 3)

    if "nc" not in _cache:
        _cache["nc"] = _build_bass()
    nc = _cache["nc"]

    in_maps = [_prep_core_inputs(template, source, c) for c in range(N_CORES)]
    res = run_bass_kernel_spmd(nc, in_maps, core_ids=list(range(N_CORES)),
                               trace=trace)

    rows = np.stack([np.asarray(r["out_row"], np.float64)
                     for r in res.results])  # [8, 128, 32] raw d2 rowmins
    cols = np.stack([np.asarray(r["out_col"], np.float64)
                     for r in res.results])  # [8, 128, 64]
    per_core_row = np.sqrt(np.maximum(rows, 0.0)).sum(axis=(1, 2))  # [8]
    cost01 = per_core_row.reshape(B, 2).sum(axis=1) / N  # [B]
    colmin = np.minimum(cols[0::2], cols[1::2])  # [B, 128, 64] raw d2
    cost10 = np.sqrt(np.maximum(colmin, 0.0)).mean(axis=(1, 2))  # [B]
    chamfer = ((cost01 + cost10) / 2.0).mean()
    return np.asarray(chamfer, dtype=np.float32), res


def kernel(template, source):
    val, _ = _run(template, source, trace=False)
    return val

